# revision 1
# baseline (speedup 1.0000x reference)
"""Online Normalization forward (nn_Norm1d) on 8 Trainium2 NeuronCores.

Reference recurrence over the batch dim t (per feature, sequential):
    d_t   = x_t - mu^{(t)}
    y_t   = d_t / sqrt(var^{(t)} + eps)
    mu^{(t+1)}  = a*mu^{(t)}  + (1-a)*x_t
    var^{(t+1)} = a*var^{(t)} + a*(1-a)*d_t^2

Sharding: tensor-parallel over the feature dim L (4096 -> 8 x 512); each
feature's scan over N=8192 is independent, so no cross-core communication.

Per-core algorithm: time lives on SBUF partitions (B=128 steps per block,
64 blocks), features on the free dim.  Constant triangular matrices on the
tensor engine evaluate a whole block of the recurrence in one matmul:

  psum_d = WD^T @ x_blk (+) CD^T @ zmu     -> rows k = d_k directly
  psum_cmu = WCX^T @ x_blk                 -> x-part of the next mu carry
  psum_v = TV^T @ e_blk (+) CVI^T @ zv     -> rows k = var^{(k)} (pre-update)
  psum_cv = TVC^T @ e_blk                  -> e-part of the next var carry
  e = Square(psum_d)  [scalar engine],  rs = Abs_reciprocal_sqrt(psum_v+eps)
  carry update (vector engine): c' = a^128 * c + psum_c   [1, L] rows
  y = psum_d * rs -> DMA out

Hardware notes baked into the structure:
  - Compute-engine access patterns must start at partition 0/32/64/96, so
    carries live in row 0 of [128, L] "ztiles" (rows 1..127 zeroed once);
    the carry-inject matmuls use a stationary whose only nonzero row is
    row 0 (K=128 streams run at full rate; K=1 injects are 2x slower).
  - Only full-128-partition DMAs engage the 16-engine descriptor spray
    (~220 GB/s vs ~25 GB/s), hence B=128 and [128, 512] transfers.
  - x is loaded via the gpsimd SWDGE ring with a free fp32->fp16 cast;
    y is stored on the sync HWDGE ring so the two streams run in parallel.
  - Rsqrt/Reciprocal activations are banned for accuracy; the
    Abs_reciprocal_sqrt table measures ~6e-4 max rel error, fine here.
"""

import sys

for _p in ("/opt/trn_rl_repo", "/root/.axon_site/_ro/trn_rl_repo"):
    if _p not in sys.path:
        sys.path.append(_p)

import numpy as np

import concourse.bacc as bacc
import concourse.mybir as mybir
from concourse.tile import TileContext
from concourse import bass_utils

N_ROWS = 8192
L_FULL = 4096
N_CORES = 8
L_SHARD = L_FULL // N_CORES

AFWD = 0.999
EPS = 1e-05
B = 128  # time steps per block

F32 = mybir.dt.float32
F16 = mybir.dt.float16
AF = mybir.ActivationFunctionType
ALU = mybir.AluOpType


def _build_weights():
    A = AFWD
    WD = np.zeros((B, B), dtype=np.float64)
    for k in range(B):
        WD[k, k] += 1.0
        for j in range(k):
            WD[j, k] -= (1 - A) * A ** (k - 1 - j)
    CD = np.zeros((B, B), dtype=np.float64)
    CD[0, :] = [-(A ** k) for k in range(B)]
    WCX = np.array([[(1 - A) * A ** (B - 1 - j)] for j in range(B)],
                   dtype=np.float64)
    TV = np.zeros((B, B), dtype=np.float64)
    for k in range(B):
        for j in range(k):
            TV[j, k] = A * (1 - A) * A ** (k - 1 - j)
    CVI = np.zeros((B, B), dtype=np.float64)
    CVI[0, :] = [A ** k for k in range(B)]
    TVC = np.array([[A * (1 - A) * A ** (B - 1 - j)] for j in range(B)],
                   dtype=np.float64)
    return {"wd": WD, "cd": CD, "wcx": WCX, "tv": TV, "cvi": CVI, "tvc": TVC}


_WEIGHTS = {k: np.ascontiguousarray(v.astype(np.float16))
            for k, v in _build_weights().items()}
A_POW_B = float(AFWD ** B)


def _build_nc(n_rows: int, l_cols: int):
    assert n_rows % B == 0
    n_blocks = n_rows // B

    nc = bacc.Bacc()
    x = nc.declare_dram_parameter("x", [n_rows, l_cols], F32, isOutput=False)
    mu0 = nc.declare_dram_parameter("mu0", [1, l_cols], F32, isOutput=False)
    var0 = nc.declare_dram_parameter("var0", [1, l_cols], F32, isOutput=False)
    wts = {
        name: nc.declare_dram_parameter(name, list(w.shape), F16, isOutput=False)
        for name, w in _WEIGHTS.items()
    }
    y = nc.declare_dram_parameter("y", [n_rows, l_cols], F32, isOutput=True)

    with TileContext(nc) as tc:
        with (
            tc.tile_pool(name="consts", bufs=1) as cpool,
            tc.tile_pool(name="mov", bufs=6) as mov_pool,
            tc.tile_pool(name="carry", bufs=3) as carry_pool,
            tc.tile_pool(name="work", bufs=4) as work_pool,
            tc.tile_pool(name="psum_big", bufs=3, space="PSUM") as psb,
            tc.tile_pool(name="psum_carry", bufs=1, space="PSUM") as psc,
        ):
            wsb = {}
            for name, w in _WEIGHTS.items():
                wsb[name] = cpool.tile(list(w.shape), F16, tag=name,
                                       name=f"w_{name}")
                nc.sync.dma_start(out=wsb[name][:, :], in_=wts[name][:, :])
            eps_sb = cpool.tile([128, 1], F32, tag="eps")
            nc.vector.memset(eps_sb[:, :], EPS)

            NZ = 3
            zmu = [carry_pool.tile([B, l_cols], F16, tag=f"zmu{i}",
                                   name=f"zmu{i}", bufs=1) for i in range(NZ)]
            zv = [carry_pool.tile([B, l_cols], F16, tag=f"zv{i}",
                                  name=f"zv{i}", bufs=1) for i in range(NZ)]
            for i in range(NZ):
                nc.vector.memset(zmu[i][:, :], 0.0)
                nc.vector.memset(zv[i][:, :], 0.0)
            nc.gpsimd.dma_start(out=zmu[0][0:1, :], in_=mu0[:, :])
            nc.gpsimd.dma_start(out=zv[0][0:1, :], in_=var0[:, :])

            for bi0 in range(0, n_blocks, 2):
                pair = [bi0, bi0 + 1]
                xts, psds, ets, psvs = {}, {}, {}, {}

                for bi in pair:
                    xt = mov_pool.tile([B, l_cols], F16, tag="xt")
                    nc.gpsimd.dma_start(out=xt[:, :],
                                        in_=x[bi * B:(bi + 1) * B, :])
                    xts[bi] = xt

                for bi in pair:
                    psum_d = psb.tile([B, l_cols], F32, tag="psum_d")
                    nc.tensor.matmul(psum_d[:, :], wsb["wd"][:, :],
                                     xts[bi][:, :], start=True, stop=False)
                    psds[bi] = psum_d
                for bi in pair:
                    if bi < n_blocks - 1:
                        psum_cmu = psc.tile([1, l_cols], F32, tag="psum_cmu")
                        nc.tensor.matmul(psum_cmu[0:1, :], wsb["wcx"][:, :],
                                         xts[bi][:, :], start=True, stop=True)
                        nc.vector.scalar_tensor_tensor(
                            zmu[(bi + 1) % NZ][0:1, :], zmu[bi % NZ][0:1, :],
                            A_POW_B, psum_cmu[0:1, :], ALU.mult, ALU.add)
                for bi in pair:
                    nc.tensor.matmul(psds[bi][:, :], wsb["cd"][:, :],
                                     zmu[bi % NZ][:, :], start=False, stop=True)

                for bi in pair:
                    et = mov_pool.tile([B, l_cols], F16, tag="et")
                    nc.scalar.square(et[:, :], psds[bi][:, :])
                    ets[bi] = et
                for bi in pair:
                    psum_v = psb.tile([B, l_cols], F32, tag="psum_v")
                    nc.tensor.matmul(psum_v[:, :], wsb["tv"][:, :],
                                     ets[bi][:, :], start=True, stop=False)
                    psvs[bi] = psum_v
                for bi in pair:
                    if bi < n_blocks - 1:
                        psum_cv = psc.tile([1, l_cols], F32, tag="psum_cv")
                        nc.tensor.matmul(psum_cv[0:1, :], wsb["tvc"][:, :],
                                         ets[bi][:, :], start=True, stop=True)
                        nc.vector.scalar_tensor_tensor(
                            zv[(bi + 1) % NZ][0:1, :], zv[bi % NZ][0:1, :],
                            A_POW_B, psum_cv[0:1, :], ALU.mult, ALU.add)
                for bi in pair:
                    nc.tensor.matmul(psvs[bi][:, :], wsb["cvi"][:, :],
                                     zv[bi % NZ][:, :], start=False, stop=True)

                for bi in pair:
                    rs = work_pool.tile([B, l_cols], F16, tag="rs")
                    nc.scalar.activation(rs[:, :], psvs[bi][:, :],
                                         AF.Abs_reciprocal_sqrt,
                                         bias=eps_sb[:, :])
                    yt = work_pool.tile([B, l_cols], F32, tag="yt")
                    nc.vector.tensor_mul(yt[:, :], psds[bi][:, :], rs[:, :])
                    nc.sync.dma_start(out=y[bi * B:(bi + 1) * B, :],
                                      in_=yt[:, :])

    nc.compile()
    return nc


_NC_CACHE = {}


def _get_nc():
    key = (N_ROWS, L_SHARD)
    if key not in _NC_CACHE:
        _NC_CACHE[key] = _build_nc(*key)
    return _NC_CACHE[key]


def kernel(x, mu0, var0, _want_time=False, _trace=False):
    x = np.ascontiguousarray(np.asarray(x), dtype=np.float32)
    mu0 = np.ascontiguousarray(np.asarray(mu0), dtype=np.float32).reshape(1, -1)
    var0 = np.ascontiguousarray(np.asarray(var0), dtype=np.float32).reshape(1, -1)
    assert x.shape == (N_ROWS, L_FULL), x.shape

    nc = _get_nc()
    in_maps = []
    for c in range(N_CORES):
        sl = slice(c * L_SHARD, (c + 1) * L_SHARD)
        in_maps.append({
            "x": np.ascontiguousarray(x[:, sl]),
            "mu0": np.ascontiguousarray(mu0[:, sl]),
            "var0": np.ascontiguousarray(var0[:, sl]),
            **_WEIGHTS,
        })

    exec_ns = None
    if _trace:
        orig_upload = bass_utils.upload_artifacts
        bass_utils.upload_artifacts = lambda tmpdir: "(skipped)"
        try:
            res = bass_utils.run_bass_kernel_spmd(
                nc, in_maps, list(range(N_CORES)), trace=True
            )
            exec_ns = res.exec_time_ns
        finally:
            bass_utils.upload_artifacts = orig_upload
    else:
        res = bass_utils.run_bass_kernel_spmd(nc, in_maps, list(range(N_CORES)))

    out = np.concatenate(
        [res.results[c]["y"] for c in range(N_CORES)], axis=1
    ).astype(np.float32, copy=False)
    if _want_time:
        return out, exec_ns
    return out

